# revision 4
# baseline (speedup 1.0000x reference)
"""Sparse KV block gather on 8 Trainium2 NeuronCores.

Problem: kv (32, 2, 64, 49, 256) f32 -> kv_flat (32, 128, 49*256);
out[b, q, k] = kv_flat[b, r_idx[b, q, k]]  -> (32, 64, 8, 49, 256).

Sharding: batch dim n=32 split across 8 cores (4 batches/core), fully
independent per-core gathers, no communication.

Per core: for each batch, 4x dma_gather (SWDGE) of 128 blocks x 50176 B
from HBM kv into SBUF (one block per partition), then HWDGE write of the
6.4 MB chunk to the contiguous output region.  Double buffered so the
gather (read stream) and write stream overlap.
"""

import numpy as np

import concourse.bacc as bacc
import concourse.bass as bass
import concourse.mybir as mybir
from concourse._compat import get_trn_type
from concourse.bass_utils import run_bass_kernel_spmd
from concourse.library_config import mlp

# Problem shapes (hardcoded per contract: kernel.py is self-contained).
N, V, P2, W2, CKV = 32, 2, 64, 49, 256
TOPK = 8
NCORES = 8
NB = N // NCORES            # 4 batches per core
BLOCKS = V * P2             # 128 source blocks per batch
ELEM = W2 * CKV             # 12544 f32 per block (50176 B)
IDX_PER_B = P2 * TOPK       # 512 gathered blocks per batch
CHUNK = 128                 # indices per dma_gather call
NCHUNK = IDX_PER_B // CHUNK  # 4 chunks per batch
TILES = NB * NCHUNK         # 16 gather+write steps per core
DEPTH = 3                   # SBUF tile buffers

_CACHE = {}


def _build_nc():
    nc = bacc.Bacc(get_trn_type() or "TRN2")
    kv_in = nc.dram_tensor(
        "kv", [NB, BLOCKS, ELEM], mybir.dt.float32, kind="ExternalInput"
    )
    idx_in = nc.dram_tensor(
        "idx", [128, TILES * (CHUNK // 16)], mybir.dt.int16, kind="ExternalInput"
    )
    out = nc.dram_tensor(
        "out", [NB, NCHUNK, CHUNK, ELEM], mybir.dt.float32, kind="ExternalOutput"
    )
    IW = CHUNK // 16  # idx columns per tile step

    with (
        nc.sbuf_tensor("tiles", [128, DEPTH, ELEM], mybir.dt.float32) as tiles,
        nc.sbuf_tensor("idx_sb", [128, TILES * IW], mybir.dt.int16) as idx_sb,
        nc.semaphore("sem_idx") as sem_idx,
        nc.semaphore("sem_g") as sem_g,
        nc.semaphore("sem_w") as sem_w,
        nc.Block() as block,
    ):

        @block.gpsimd
        def _(gpsimd):
            gpsimd.load_library(mlp)
            gpsimd.dma_start(out=idx_sb[:], in_=idx_in[:]).then_inc(sem_idx, 16)
            gpsimd.wait_ge(sem_idx, 16)
            for t in range(TILES):
                n = t // NCHUNK
                buf = t % DEPTH
                if t >= DEPTH:
                    # wait until the writer has drained this buffer
                    gpsimd.wait_ge(sem_w, (t - DEPTH + 1) * 16)
                gpsimd.dma_gather(
                    tiles[:, buf : buf + 1, :],
                    kv_in[n],
                    idx_sb[:, t * IW : (t + 1) * IW],
                    num_idxs=CHUNK,
                    num_idxs_reg=CHUNK,
                    elem_size=ELEM,
                ).then_inc(sem_g, 16)

        @block.sync
        def _(sync):
            for t in range(TILES):
                n, c = divmod(t, NCHUNK)
                buf = t % DEPTH
                sync.wait_ge(sem_g, (t + 1) * 16)
                sync.dma_start(out=out[n, c], in_=tiles[:, buf, :]).then_inc(
                    sem_w, 16
                )
            sync.wait_ge(sem_w, TILES * 16)

    nc.compile()
    return nc


def _prep_idx(r_idx_core: np.ndarray) -> np.ndarray:
    """r_idx_core: (NB, P2, TOPK) int -> (128, TILES * CHUNK//16) int16.

    dma_gather reads index k of a call from partition k%16, column k//16,
    replicated across each group of 16 partitions.
    """
    flat = r_idx_core.reshape(NB * NCHUNK, CHUNK).astype(np.int16)
    # position i within a chunk -> [i % 16, i // 16]
    wrapped = flat.reshape(TILES, CHUNK // 16, 16).transpose(0, 2, 1)  # (T,16,IW)
    cols = wrapped.transpose(1, 0, 2).reshape(16, TILES * (CHUNK // 16))
    return np.tile(cols, (8, 1)).copy()  # replicate to all 128 partitions


def kernel(r_idx: np.ndarray, r_weight: np.ndarray, kv: np.ndarray) -> np.ndarray:
    if "nc" not in _CACHE:
        _CACHE["nc"] = _build_nc()
    nc = _CACHE["nc"]

    kv_r = np.ascontiguousarray(kv.reshape(N, BLOCKS, ELEM), dtype=np.float32)
    in_maps = []
    for c in range(NCORES):
        lo = c * NB
        in_maps.append(
            {
                "kv": kv_r[lo : lo + NB],
                "idx": _prep_idx(np.asarray(r_idx)[lo : lo + NB]),
            }
        )

    res = run_bass_kernel_spmd(nc, in_maps, core_ids=list(range(NCORES)))
    outs = [res.results[c]["out"].reshape(NB, P2, TOPK, W2, CKV) for c in range(NCORES)]
    return np.concatenate(outs, axis=0)


# revision 8
# speedup vs baseline: 1.3947x; 1.3947x over previous
"""Sparse KV block gather on 8 Trainium2 NeuronCores.

Problem: kv (32, 2, 64, 49, 256) f32 -> kv_flat (32, 128, 49*256);
out[b, q, k] = kv_flat[b, r_idx[b, q, k]]  -> (32, 64, 8, 49, 256).

Sharding: batch dim n=32 split across 8 cores (4 batches/core).

Strategy (v2, read-deduplicated): each batch's kv (6.4 MB = 128 blocks x
50 KB) is staged once in SBUF, one block per partition.  The gather is a
dynamic partition permutation, done on TensorE as fp32 matmuls against
one-hot selection matrices (bitwise exact: 1.0*x accumulated in fp32
PSUM).  PSUM tiles are drained to SBUF by VectorE/ScalarE alternately,
then written to HBM by HWDGE DMA.  HBM traffic per core drops from
2x103 MB (gather re-reads) to 26 MB read + 103 MB write, while TensorE
(~356 us) runs concurrently with DMA (~360 us floor).

The one-hot matrices are a pure host-side re-encoding of r_idx (1 MB per
core); kv itself is shipped raw.
"""

import numpy as np

import concourse.bacc as bacc
import concourse.bass as bass
import concourse.mybir as mybir
from concourse._compat import get_trn_type
from concourse.bass_utils import run_bass_kernel_spmd

# Problem shapes (hardcoded per contract: kernel.py is self-contained).
N, V, P2, W2, CKV = 32, 2, 64, 49, 256
TOPK = 8
NCORES = 8
NB = N // NCORES             # 4 batches per core
BLOCKS = V * P2              # 128 source blocks per batch
ELEM = W2 * CKV              # 12544 f32 per block (50176 B)
IDX_PER_B = P2 * TOPK        # 512 gathered blocks per batch
JCHUNK = 128                 # output blocks per one-hot matmul group
NJC = IDX_PER_B // JCHUNK    # 4 j-chunks per batch
FT = 448                     # f-columns per matmul tile (12544 = 28*448)
NFT = ELEM // FT             # 28 tiles per j-chunk
HALF = NFT // 2              # 14 tiles per DMA-out half (6272 f32)
NT = NB * NJC * NFT          # 448 matmul tiles per core
NG = NT // HALF              # 32 DMA-out groups per core

_CACHE = {}


def _build_nc():
    nc = bacc.Bacc(get_trn_type() or "TRN2")
    kv_in = nc.dram_tensor(
        "kv", [NB, BLOCKS, ELEM], mybir.dt.float32, kind="ExternalInput"
    )
    oh_in = nc.dram_tensor(
        "oh", [128, NB * NJC * JCHUNK], mybir.dt.float32, kind="ExternalInput"
    )
    out = nc.dram_tensor(
        "out", [NB, NJC, JCHUNK, ELEM], mybir.dt.float32, kind="ExternalOutput"
    )

    with (
        nc.sbuf_tensor("kv_sb", [128, 2, ELEM], mybir.dt.float32) as kv_sb,
        nc.sbuf_tensor("oh_sb", [128, NB * NJC * JCHUNK], mybir.dt.float32) as oh_sb,
        nc.sbuf_tensor("stage", [128, 2, HALF * FT], mybir.dt.float32) as stage,
        nc.psum_tensor("ps", [128, 8, 512], mybir.dt.float32) as ps,
        nc.semaphore("s_oh") as s_oh,
        nc.semaphore("s_ld") as s_ld,
        nc.semaphore("s_mm") as s_mm,
        nc.semaphore("s_drv") as s_drv,   # DVE drains (even tiles)
        nc.semaphore("s_dra") as s_dra,   # ACT drains (odd tiles)
        nc.semaphore("s_out") as s_out,
        nc.Block() as block,
    ):

        @block.gpsimd
        def _(gpsimd):
            gpsimd.dma_start(out=oh_sb[:], in_=oh_in[:]).then_inc(s_oh, 16)
            for n in range(NB):
                if n >= 2:
                    # kv buffer n%2 reused; all matmuls of batch n-2 done
                    gpsimd.wait_ge(s_mm, (n - 1) * NJC * NFT)
                gpsimd.dma_start(
                    out=kv_sb[:, n % 2, :], in_=kv_in[n]
                ).then_inc(s_ld, 16)

        @block.tensor
        def _(tensor):
            tensor.wait_ge(s_oh, 16)
            for t in range(NT):
                n = t // (NJC * NFT)
                c = (t // NFT) % NJC
                k = t % NFT
                if k == 0 and c == 0:
                    tensor.wait_ge(s_ld, 16 * (n + 1))
                if t >= 8:
                    # PSUM bank t%8 free once drain t-8 completed
                    td = t - 8
                    if td % 2 == 0:
                        tensor.wait_ge(s_drv, td // 2 + 1)
                    else:
                        tensor.wait_ge(s_dra, td // 2 + 1)
                tensor.matmul(
                    ps[:, t % 8, 0:FT],
                    oh_sb[:, (n * NJC + c) * JCHUNK : (n * NJC + c + 1) * JCHUNK],
                    kv_sb[:, n % 2, k * FT : (k + 1) * FT],
                    start=True,
                    stop=True,
                ).then_inc(s_mm, 1)

        def _drain(eng, parity, sem):
            for t in range(parity, NT, 2):
                g = t // HALF
                kk = t % HALF
                eng.wait_ge(s_mm, t + 1)
                if g >= 2:
                    # stage slot g%2 free once DMA-out g-2 issued+done
                    eng.wait_ge(s_out, 16 * (g - 1))
                eng_copy = (
                    eng.tensor_copy if parity == 0 else eng.copy
                )
                eng_copy(
                    stage[:, g % 2, kk * FT : (kk + 1) * FT],
                    ps[:, t % 8, 0:FT],
                ).then_inc(sem, 1)

        @block.vector
        def _(vector):
            _drain(vector, 0, s_drv)

        @block.scalar
        def _(scalar):
            _drain(scalar, 1, s_dra)

        @block.sync
        def _(sync):
            for g in range(NG):
                t0 = g * HALF
                n = t0 // (NJC * NFT)
                c = (t0 // NFT) % NJC
                h = (t0 % NFT) // HALF
                # drains of tiles t0..t0+HALF-1 must have completed
                sync.wait_ge(s_drv, (t0 + HALF + 1) // 2)
                sync.wait_ge(s_dra, (t0 + HALF) // 2)
                sync.dma_start(
                    out=out[n, c, :, h * HALF * FT : (h + 1) * HALF * FT],
                    in_=stage[:, g % 2, :],
                ).then_inc(s_out, 16)
            sync.wait_ge(s_out, 16 * NG)

    nc.compile()
    return nc


def _prep_onehot(r_idx_core: np.ndarray) -> np.ndarray:
    """r_idx_core: (NB, P2, TOPK) -> one-hot lhsT in SBUF layout
    (128, NB*NJC*JCHUNK) f32:  arr[i, g*128 + j] = 1 iff r_idx_flat[g, j] == i.
    """
    idx = r_idx_core.reshape(NB * NJC, JCHUNK).astype(np.int64)
    oh = np.zeros((NB * NJC, 128, JCHUNK), np.float32)
    g = np.arange(NB * NJC)[:, None]
    j = np.arange(JCHUNK)[None, :]
    oh[g, idx, j] = 1.0
    return np.ascontiguousarray(oh.transpose(1, 0, 2).reshape(128, NB * NJC * JCHUNK))


def make_in_maps(r_idx: np.ndarray, kv: np.ndarray) -> list:
    kv_r = np.ascontiguousarray(kv.reshape(N, BLOCKS, ELEM), dtype=np.float32)
    in_maps = []
    for c in range(NCORES):
        lo = c * NB
        in_maps.append(
            {
                "kv": kv_r[lo : lo + NB],
                "oh": _prep_onehot(np.asarray(r_idx)[lo : lo + NB]),
            }
        )
    return in_maps


def kernel(r_idx: np.ndarray, r_weight: np.ndarray, kv: np.ndarray) -> np.ndarray:
    if "nc" not in _CACHE:
        _CACHE["nc"] = _build_nc()
    nc = _CACHE["nc"]

    in_maps = make_in_maps(r_idx, kv)
    res = run_bass_kernel_spmd(nc, in_maps, core_ids=list(range(NCORES)))
    outs = [res.results[c]["out"].reshape(NB, P2, TOPK, W2, CKV) for c in range(NCORES)]
    return np.concatenate(outs, axis=0)


# revision 10
# speedup vs baseline: 1.5462x; 1.1086x over previous
"""Sparse KV block gather on 8 Trainium2 NeuronCores.

Problem: kv (32, 2, 64, 49, 256) f32 -> kv_flat (32, 128, 49*256);
out[b, q, k] = kv_flat[b, r_idx[b, q, k]]  -> (32, 64, 8, 49, 256).

Sharding: batch dim n=32 split across 8 cores (4 batches/core).

Strategy (v2, read-deduplicated): each batch's kv (6.4 MB = 128 blocks x
50 KB) is staged once in SBUF, one block per partition.  The gather is a
dynamic partition permutation, done on TensorE as fp32 matmuls against
one-hot selection matrices (bitwise exact: 1.0*x accumulated in fp32
PSUM).  PSUM tiles are drained to SBUF by VectorE/ScalarE alternately,
then written to HBM by HWDGE DMA.  HBM traffic per core drops from
2x103 MB (gather re-reads) to 26 MB read + 103 MB write, while TensorE
(~356 us) runs concurrently with DMA (~360 us floor).

The one-hot matrices are a pure host-side re-encoding of r_idx (1 MB per
core); kv itself is shipped raw.
"""

import numpy as np

import concourse.bacc as bacc
import concourse.bass as bass
import concourse.mybir as mybir
from concourse._compat import get_trn_type
from concourse.bass_utils import run_bass_kernel_spmd

# Problem shapes (hardcoded per contract: kernel.py is self-contained).
N, V, P2, W2, CKV = 32, 2, 64, 49, 256
TOPK = 8
NCORES = 8
NB = N // NCORES             # 4 batches per core
BLOCKS = V * P2              # 128 source blocks per batch
ELEM = W2 * CKV              # 12544 f32 per block (50176 B)
IDX_PER_B = P2 * TOPK        # 512 gathered blocks per batch
JCHUNK = 128                 # output blocks per one-hot matmul group
NJC = IDX_PER_B // JCHUNK    # 4 j-chunks per batch
FT = 448                     # f-columns per matmul tile (12544 = 28*448)
NFT = ELEM // FT             # 28 tiles per j-chunk
HALF = NFT // 2              # 14 tiles per DMA-out half (6272 f32)
NT = NB * NJC * NFT          # 448 matmul tiles per core
NG = NT // HALF              # 32 DMA-out groups per core

_CACHE = {}


def _build_nc():
    nc = bacc.Bacc(get_trn_type() or "TRN2")
    kv_in = nc.dram_tensor(
        "kv", [NB, BLOCKS, ELEM], mybir.dt.float32, kind="ExternalInput"
    )
    oh_in = nc.dram_tensor(
        "oh", [128, NB * NJC * JCHUNK], mybir.dt.float32, kind="ExternalInput"
    )
    out = nc.dram_tensor(
        "out", [NB, NJC, JCHUNK, ELEM], mybir.dt.float32, kind="ExternalOutput"
    )

    with (
        nc.sbuf_tensor("kv_sb", [128, 2, ELEM], mybir.dt.float32) as kv_sb,
        nc.sbuf_tensor("oh_sb", [128, NB * NJC * JCHUNK], mybir.dt.float32) as oh_sb,
        nc.sbuf_tensor("stage", [128, 2, HALF * FT], mybir.dt.float32) as stage,
        nc.psum_tensor("ps", [128, 8, 512], mybir.dt.float32) as ps,
        nc.semaphore("s_oh") as s_oh,
        nc.semaphore("s_ld") as s_ld,
        nc.semaphore("s_mm") as s_mm,
        nc.semaphore("s_drv") as s_drv,   # DVE drains (even tiles)
        nc.semaphore("s_dra") as s_dra,   # ACT drains (odd tiles)
        nc.semaphore("s_out") as s_out,
        nc.Block() as block,
    ):

        QF = ELEM // 4        # 3136 f32 per quarter load
        QK = NFT // 4         # 7 matmul tiles per quarter

        @block.gpsimd
        def _(gpsimd):
            # kv loads at f-quarter granularity: batch n's quarter q is
            # last read by matmul t = n*112 + 84 + 7q + 6, so the load
            # for batch n+2 can start early, well before the boundary.
            for n in range(NB):
                for q in range(4):
                    if n >= 2:
                        gpsimd.wait_ge(
                            s_mm, (n - 2) * NJC * NFT + 3 * NFT + QK * q + QK
                        )
                    gpsimd.dma_start(
                        out=kv_sb[:, n % 2, q * QF : (q + 1) * QF],
                        in_=kv_in[n][:, q * QF : (q + 1) * QF],
                    ).then_inc(s_ld, 16)

        @block.tensor
        def _(tensor):
            tensor.wait_ge(s_oh, 16)
            for t in range(NT):
                n = t // (NJC * NFT)
                c = (t // NFT) % NJC
                k = t % NFT
                if c == 0 and k % QK == 0:
                    tensor.wait_ge(s_ld, 16 * (4 * n + k // QK + 1))
                if t >= 8:
                    # PSUM bank t%8 free once drain t-8 completed
                    td = t - 8
                    if td % 2 == 0:
                        tensor.wait_ge(s_drv, td // 2 + 1)
                    else:
                        tensor.wait_ge(s_dra, td // 2 + 1)
                tensor.matmul(
                    ps[:, t % 8, 0:FT],
                    oh_sb[:, (n * NJC + c) * JCHUNK : (n * NJC + c + 1) * JCHUNK],
                    kv_sb[:, n % 2, k * FT : (k + 1) * FT],
                    start=True,
                    stop=True,
                ).then_inc(s_mm, 1)

        def _drain(eng, parity, sem):
            for t in range(parity, NT, 2):
                g = t // HALF
                kk = t % HALF
                eng.wait_ge(s_mm, t + 1)
                if g >= 2:
                    # stage slot g%2 free once DMA-out g-2 issued+done
                    eng.wait_ge(s_out, 16 * (g - 1))
                eng_copy = (
                    eng.tensor_copy if parity == 0 else eng.copy
                )
                eng_copy(
                    stage[:, g % 2, kk * FT : (kk + 1) * FT],
                    ps[:, t % 8, 0:FT],
                ).then_inc(sem, 1)

        @block.vector
        def _(vector):
            _drain(vector, 0, s_drv)

        @block.scalar
        def _(scalar):
            _drain(scalar, 1, s_dra)

        @block.sync
        def _(sync):
            sync.dma_start(out=oh_sb[:], in_=oh_in[:]).then_inc(s_oh, 16)
            for g in range(NG):
                t0 = g * HALF
                n = t0 // (NJC * NFT)
                c = (t0 // NFT) % NJC
                h = (t0 % NFT) // HALF
                # drains of tiles t0..t0+HALF-1 must have completed
                sync.wait_ge(s_drv, (t0 + HALF + 1) // 2)
                sync.wait_ge(s_dra, (t0 + HALF) // 2)
                sync.dma_start(
                    out=out[n, c, :, h * HALF * FT : (h + 1) * HALF * FT],
                    in_=stage[:, g % 2, :],
                ).then_inc(s_out, 16)
            sync.wait_ge(s_out, 16 * NG)

    nc.compile()
    return nc


def _prep_onehot(r_idx_core: np.ndarray) -> np.ndarray:
    """r_idx_core: (NB, P2, TOPK) -> one-hot lhsT in SBUF layout
    (128, NB*NJC*JCHUNK) f32:  arr[i, g*128 + j] = 1 iff r_idx_flat[g, j] == i.
    """
    idx = r_idx_core.reshape(NB * NJC, JCHUNK).astype(np.int64)
    oh = np.zeros((NB * NJC, 128, JCHUNK), np.float32)
    g = np.arange(NB * NJC)[:, None]
    j = np.arange(JCHUNK)[None, :]
    oh[g, idx, j] = 1.0
    return np.ascontiguousarray(oh.transpose(1, 0, 2).reshape(128, NB * NJC * JCHUNK))


def make_in_maps(r_idx: np.ndarray, kv: np.ndarray) -> list:
    kv_r = np.ascontiguousarray(kv.reshape(N, BLOCKS, ELEM), dtype=np.float32)
    in_maps = []
    for c in range(NCORES):
        lo = c * NB
        in_maps.append(
            {
                "kv": kv_r[lo : lo + NB],
                "oh": _prep_onehot(np.asarray(r_idx)[lo : lo + NB]),
            }
        )
    return in_maps


def kernel(r_idx: np.ndarray, r_weight: np.ndarray, kv: np.ndarray) -> np.ndarray:
    if "nc" not in _CACHE:
        _CACHE["nc"] = _build_nc()
    nc = _CACHE["nc"]

    in_maps = make_in_maps(r_idx, kv)
    res = run_bass_kernel_spmd(nc, in_maps, core_ids=list(range(NCORES)))
    outs = [res.results[c]["out"].reshape(NB, P2, TOPK, W2, CKV) for c in range(NCORES)]
    return np.concatenate(outs, axis=0)
